# revision 5
# baseline (speedup 1.0000x reference)
"""Multi-head attention (B=4, S=2048, D=1024, H=16) on 8 TRN2 NeuronCores.

Sharding: core c -> (batch b = c//2, head-group g = c%2 of 8 heads).
Data parallel over batch, tensor parallel over heads; host sums the two
per-batch partials and adds the V-bias epilogue.

All GEMMs in f16 (fp8 fails the 2e-2 accuracy gate: every fp8-quantized
tensor alone contributes ~2-3e-2). Causal masking is applied as an
additive -1e9 triangle f32 bias on the PSUM logits BEFORE the exp (one
shared [128,128] tile serves every diagonal block), so there are no
multiplicative mask strips and stair blocks get exact per-block q0
trims (widths 512/384/256/128). The softmax denominator rides the PV
matmul as a leading ones-column of V; normalization is reciprocal +
K=1 broadcast matmul + DVE multiply, batched per head-pair.

Scheduling: TensorE executes in program order; QKV-projection pieces
for the next seq block and output-projection pieces for the previous
q-tile are woven between attention blocks so the PE stays busy while
the softmax EXP runs on ScalarE.
"""

import os
import numpy as np

B, S, D, H = 4, 2048, 1024, 16
DK = D // H          # 64
HPC = H // 2         # heads per core = 8
GD = HPC * DK        # group feature width = 512
QT = 512             # q-tile width
KTL = 128            # k-tile length (partition dim of S.T blocks)
N_QT = S // QT       # 4
N_KT = S // KTL      # 16
ND = D // 128        # 8 contraction chunks
NM = 8               # qk projection output chunks (2*GD/128)
NEG = np.float32(-1e9)
SCALE = float(1.0 / np.sqrt(np.float32(DK)))

_cache = {}
last_results = None


def _build():
    import concourse.bacc as bacc
    import concourse.tile as tile
    import concourse.mybir as mybir
    from contextlib import ExitStack

    f32 = mybir.dt.float32
    f16 = mybir.dt.float16
    Exp = mybir.ActivationFunctionType.Exp

    nc = bacc.Bacc(trn_type="TRN2", target_bir_lowering=False, debug=False)
    xT = nc.dram_tensor("xT", [D, S], f16, kind="ExternalInput").ap()
    w_qk = nc.dram_tensor("w_qk", [D, 2 * GD], f16, kind="ExternalInput").ap()
    b_qk = nc.dram_tensor("b_qk", [2 * GD], f32, kind="ExternalInput").ap()
    w_v = nc.dram_tensor("w_v", [D, GD], f16, kind="ExternalInput").ap()
    wo_T = nc.dram_tensor("wo_T", [GD, D], f16, kind="ExternalInput").ap()
    trid = nc.dram_tensor("trid", [KTL, KTL], f32, kind="ExternalInput").ap()
    outT16 = nc.dram_tensor("outT16", [D, S], f16, kind="ExternalOutput").ap()

    with tile.TileContext(nc) as tc, ExitStack() as ctx:
        singles = ctx.enter_context(tc.tile_pool(name="singles", bufs=1))
        qkt_pool = ctx.enter_context(tc.tile_pool(name="qkt", bufs=1))
        pt_pool = ctx.enter_context(tc.tile_pool(name="pt", bufs=3))
        nrm_pool = ctx.enter_context(tc.tile_pool(name="nrm", bufs=2))
        otq_pool = ctx.enter_context(tc.tile_pool(name="otq", bufs=2))
        ob_pool = ctx.enter_context(tc.tile_pool(name="ob", bufs=4))

        # Q.T / K.T in f16: chunks 0-3 = Q (512 feats), 4-7 = K
        qkt = [qkt_pool.tile([128, S], f16, tag=f"qkt{m}", name=f"qkt{m}")
               for m in range(NM)]
        # V per kt: [128 k, HPC, 128]; col 0 = ones (denominator), 64:128 = V
        v16 = [v for t in range(N_KT)
               for v in [singles.tile([128, HPC, 128], f16, tag=f"v{t}",
                                      name=f"v{t}")]]
        xs = [singles.tile([128, S], f16, tag=f"xs{k}", name=f"xs{k}")
              for k in range(ND)]
        wqk_t = [singles.tile([128, 2 * GD], f16, tag=f"wqk{k}",
                              name=f"wqk{k}") for k in range(ND)]
        wv_t = [singles.tile([128, GD], f16, tag=f"wv{k}", name=f"wv{k}")
                for k in range(ND)]
        wo_t = [singles.tile([128, D], f16, tag=f"wo{k}", name=f"wo{k}")
                for k in range(4)]
        bqk_t = singles.tile([128, NM], f32)
        tri = singles.tile([KTL, KTL], f32)
        ones_col = singles.tile([1, DK], f16)

        # DMA order: operands of the first projection matmuls first.
        for k in range(ND):
            nc.sync.dma_start(out=xs[k][:, 0:QT], in_=xT[128 * k:128 * (k + 1),
                                                         0:QT])
            nc.sync.dma_start(out=wqk_t[k], in_=w_qk[128 * k:128 * (k + 1)])
        nc.sync.dma_start(out=bqk_t, in_=b_qk.rearrange("(m p) -> p m", p=128))
        for k in range(ND):
            nc.sync.dma_start(out=wv_t[k], in_=w_v[128 * k:128 * (k + 1)])
        for sb in range(1, N_QT):
            for k in range(ND):
                nc.sync.dma_start(out=xs[k][:, QT * sb:QT * (sb + 1)],
                                  in_=xT[128 * k:128 * (k + 1),
                                         QT * sb:QT * (sb + 1)])
        for k in range(4):
            nc.sync.dma_start(out=wo_t[k], in_=wo_T[128 * k:128 * (k + 1)])
        nc.sync.dma_start(out=tri, in_=trid)
        nc.vector.memset(ones_col, 1.0)
        for t in range(N_KT):
            nc.vector.memset(v16[t][:, :, 0:1], 1.0)
            nc.vector.memset(v16[t][:, :, 1:64], 0.0)

        # ==== projection for seq block 0: transient 8-bank k-outer layout ====
        with tc.tile_pool(name="p1ps0", bufs=8, space="PSUM") as p1ps0:
            pss = [p1ps0.tile([128, QT], f32, tag="p10", name=f"ps0{m}")
                   for m in range(NM)]
            for k in range(ND):
                for m in range(NM):
                    nc.tensor.matmul(
                        pss[m][:], wqk_t[k][:, 128 * m:128 * (m + 1)],
                        xs[k][:, 0:QT], start=(k == 0), stop=(k == ND - 1))
            for m in range(NM):
                nc.vector.tensor_scalar_add(qkt[m][:, 0:QT], pss[m][:],
                                            bqk_t[:, m:m + 1])
            for t in range(4):
                ps = p1ps0.tile([128, GD], f32, tag="p10", name="ps0_v")
                for k in range(ND):
                    nc.tensor.matmul(
                        ps[:], xs[k][:, 128 * t:128 * (t + 1)], wv_t[k][:],
                        start=(k == 0), stop=(k == ND - 1))
                nc.vector.tensor_copy(
                    out=v16[t][:, :, 64:128],
                    in_=ps[:].rearrange("p (h d) -> p h d", h=HPC))

        # ==== steady-state pools: 4 + 2 + 2 = 8 PSUM banks ====
        st_pool = ctx.enter_context(tc.tile_pool(name="st", bufs=2,
                                                 space="PSUM"))
        ot_pool = ctx.enter_context(tc.tile_pool(name="ot", bufs=1,
                                                 space="PSUM"))
        p1p = ctx.enter_context(tc.tile_pool(name="p1p", bufs=2, space="PSUM"))

        def qk_proj_piece(m, sb):
            ps = p1p.tile([128, QT], f32, tag="p1", name="ps_qk")
            for k in range(ND):
                nc.tensor.matmul(
                    ps[:], wqk_t[k][:, 128 * m:128 * (m + 1)],
                    xs[k][:, QT * sb:QT * (sb + 1)],
                    start=(k == 0), stop=(k == ND - 1))
            nc.vector.tensor_scalar_add(
                qkt[m][:, QT * sb:QT * (sb + 1)], ps[:], bqk_t[:, m:m + 1])

        def v_proj_piece(t):
            ps = p1p.tile([128, GD], f32, tag="p1", name="ps_v")
            for k in range(ND):
                nc.tensor.matmul(
                    ps[:], xs[k][:, 128 * t:128 * (t + 1)], wv_t[k][:],
                    start=(k == 0), stop=(k == ND - 1))
            nc.vector.tensor_copy(
                out=v16[t][:, :, 64:128],
                in_=ps[:].rearrange("p (h d) -> p h d", h=HPC))

        def o_proj_piece(m, qi, otq):
            ps = p1p.tile([128, QT], f32, tag="p1", name="ps_o")
            for k in range(4):
                nc.tensor.matmul(
                    ps[:], wo_t[k][:, 128 * m:128 * (m + 1)], otq[k][:],
                    start=(k == 0), stop=(k == 3))
            ob = ob_pool.tile([128, QT], f16, tag="ob", name="ob")
            nc.vector.tensor_copy(out=ob[:], in_=ps[:])
            nc.sync.dma_start(
                out=outT16[128 * m:128 * (m + 1), QT * qi:QT * (qi + 1)],
                in_=ob[:])

        # ==== attention ====
        prev_otq = None
        for qi in range(N_QT):
            fill = []
            if qi == 0:
                fill += [(lambda m=m: qk_proj_piece(m, 1)) for m in range(NM)]
                fill += [(lambda t=t: v_proj_piece(t)) for t in range(4, 8)]
            elif qi == 1:
                fill += [(lambda m=m: qk_proj_piece(m, 2)) for m in range(NM)]
                fill += [(lambda t=t: v_proj_piece(t)) for t in range(8, 12)]
            elif qi == 2:
                fill += [(lambda m=m: qk_proj_piece(m, 3)) for m in range(NM)]
                fill += [(lambda t=t: v_proj_piece(t)) for t in range(12, 16)]
            if prev_otq is not None:
                fill += [(lambda m=m, q=qi - 1, o=prev_otq: o_proj_piece(m, q, o))
                         for m in range(NM)]

            n_kts = 4 * (qi + 1)
            n_units = n_kts * 4 * 2   # 2 weave points per block
            pace = len(fill) / n_units
            acc = 0.0
            fi = 0

            otq = [otq_pool.tile([128, QT], f16, tag=f"otq{k}",
                                 name=f"otq{k}") for k in range(4)]
            for hp in range(4):
                ot = ot_pool.tile([128, 2, QT], f32, tag="ot", name="ot")
                for kt in range(n_kts):
                    dj = 128 * (kt - (n_kts - 4))
                    q0 = max(dj, 0)          # stair blocks: exact trim
                    st = st_pool.tile([128, 2, QT], f32, tag="st", name="st")
                    for h in range(2):
                        lo, hi = 64 * h, 64 * h + 64
                        nc.tensor.matmul(
                            st[:, h, q0:QT],
                            qkt[4 + hp][lo:hi, KTL * kt:KTL * (kt + 1)],
                            qkt[hp][lo:hi, QT * qi + q0:QT * (qi + 1)],
                            start=True, stop=True, tile_position=(64 * h, 0))
                    if dj >= 0:
                        nc.vector.tensor_add(
                            st[:, :, dj:dj + 128], st[:, :, dj:dj + 128],
                            tri[:, None, :].broadcast_to([KTL, 2, KTL]))
                    pt = pt_pool.tile([128, 2, QT], f16, tag="pt", name="pt")
                    nc.scalar.activation(out=pt[:, :, q0:QT],
                                         in_=st[:, :, q0:QT],
                                         func=Exp, scale=SCALE)
                    acc += pace
                    while acc >= 1.0 and fi < len(fill):
                        fill[fi]()
                        fi += 1
                        acc -= 1.0
                    for h in range(2):
                        nc.tensor.matmul(
                            ot[:, h, q0:QT], v16[kt][:, 2 * hp + h, :],
                            pt[:, h, q0:QT],
                            start=(kt == 0), stop=(kt == n_kts - 1))
                    acc += pace
                    while acc >= 1.0 and fi < len(fill):
                        fill[fi]()
                        fi += 1
                        acc -= 1.0
                # softmax normalization for this head pair
                dn = nrm_pool.tile([1, 2, QT], f32, tag="dn", name="dn")
                nc.vector.tensor_copy(out=dn, in_=ot[0:1, :, :])
                rn = nrm_pool.tile([1, 2, QT], f32, tag="rn", name="rn")
                nc.vector.reciprocal_approx_fast(out=rn, in_=dn)
                r16 = nrm_pool.tile([1, 2, QT], f16, tag="r16", name="r16")
                nc.vector.tensor_copy(out=r16, in_=rn)
                rb_ps = p1p.tile([128, QT], f32, tag="p1", name="rb_ps")
                for h in range(2):
                    nc.tensor.matmul(rb_ps[64 * h:64 * h + 64, :], ones_col[:],
                                     r16[0:1, h, :], start=True, stop=True)
                rb = nrm_pool.tile([128, QT], f16, tag="rb", name="rb")
                nc.vector.tensor_copy(out=rb, in_=rb_ps)
                for h in range(2):
                    nc.vector.tensor_mul(
                        otq[hp][64 * h:64 * h + 64, :],
                        ot[64:128, h, :], rb[64 * h:64 * h + 64, :])
            while fi < len(fill):
                fill[fi]()
                fi += 1
            prev_otq = otq
        for m in range(NM):
            o_proj_piece(m, N_QT - 1, prev_otq)
    nc.compile()
    return nc


def kernel(encodings_for_qkv, mask, w_qkv, b_qkv, w_o):
    global last_results
    from concourse.bass_utils import run_bass_kernel_spmd

    x = np.asarray(encodings_for_qkv, dtype=np.float32)
    mask2d = np.asarray(mask).reshape(S, S).astype(bool)
    w_qkv = np.asarray(w_qkv, dtype=np.float32)
    b_qkv = np.asarray(b_qkv, dtype=np.float32)
    w_o = np.asarray(w_o, dtype=np.float32)

    causal = np.triu(np.ones((S, S), dtype=bool), k=1)
    assert np.array_equal(mask2d, causal), "kernel specialised for causal mask"

    if "nc" not in _cache:
        _cache["nc"] = _build()
    nc = _cache["nc"]

    wT = w_qkv.T                               # [D, 3D]
    woT_full = w_o.T                           # [D(in), D(out)]

    kl = np.arange(KTL)[:, None]
    jl = np.arange(KTL)[None, :]
    tri = np.where(kl > jl, NEG, np.float32(0)).astype(np.float32)

    in_maps = []
    for c in range(8):
        b, g = divmod(c, 2)
        cols = slice(GD * g, GD * (g + 1))
        w_qk_g = np.ascontiguousarray(np.concatenate(
            [wT[:, 0 * D:1 * D][:, cols], wT[:, 1 * D:2 * D][:, cols]],
            axis=1))
        b_qk_g = np.ascontiguousarray(np.concatenate(
            [b_qkv[0 * D:1 * D][cols], b_qkv[1 * D:2 * D][cols]]))
        w_v_g = np.ascontiguousarray(wT[:, 2 * D:3 * D][:, cols])
        wo_T_g = np.ascontiguousarray(woT_full[cols, :])
        in_maps.append({
            "xT": np.ascontiguousarray(x[b].T).astype(np.float16),
            "w_qk": w_qk_g.astype(np.float16), "b_qk": b_qk_g,
            "w_v": w_v_g.astype(np.float16),
            "wo_T": wo_T_g.astype(np.float16),
            "trid": tri,
        })

    trace = bool(int(os.environ.get("KERNEL_PROFILE", "0")))
    res = run_bass_kernel_spmd(nc, in_maps, core_ids=list(range(8)),
                               trace=trace,
                               trace_cores=list(range(8)) if trace else None)
    last_results = res

    out = np.empty((B, S, D), dtype=np.float32)
    for b in range(B):
        acc = (res.results[2 * b]["outT16"].astype(np.float32)
               + res.results[2 * b + 1]["outT16"].astype(np.float32))
        out[b] = acc.T
    # V-bias epilogue: softmax rows sum to 1, so the V bias contributes a
    # constant (b_v @ w_o.T) to every sequence position.
    out += (b_qkv[2 * D:] @ woT_full).reshape(1, 1, D)
    return out
